# revision 2
# baseline (speedup 1.0000x reference)
"""HDTimeCrystalBlock kernel for 8 Trainium2 NeuronCores.

Math: out = ((x @ W_in) * mod[None]) @ W_out, where
  mod[l,h] = sum_m coupled[m] * cos(omega*(m+1)*t[l] + E[m,h])

mod depends only on (l,h) -- a [L,HD] table -- and costs ~0.5 GFLOP to
build, so it is computed on the HOST in fp64 (cos/sin tables + two tiny
[L,32]@[32,HD] gemms) and shipped to the device as a 2 MiB bf16 input.
The device is then a pure matmul pipeline: pa = W_in^T-tile @ x-tile,
hm = pa * mod-tile (DVE), y += W_out-tile^T @ hm -- no mod matmuls, no
PSUM->SBUF mod copies, no warm-up filler. The PE starts as soon as the
first x / W_in strips land (~7us) and streams 512 back-to-back
[128x128]@[128x512] bf16 matmuls (~110us, the PE roofline).

Sharding: split L=2048 into 8 chunks of 256; each core handles its
l-chunk for ALL 4 batches (1024 tokens, b-major). mod depends only on
l, so each core loads only its own [LCH,HD] mod slice. Activations stay
transposed ([feature, token]); weights are the stationary operand.

DMA: two HWDGE rings (sync + scalar). Early transfers are small and
ordered just-in-time (xts0 per-k, w_in j0 strip first) so the first pa
group issues ~1.3us after engine start; later transfers are big. Output
stores are split across both rings by partition halves to halve the
descriptor-storm tail.
"""
import math

import numpy as np

B, L, D, HD, M = 4, 2048, 512, 4096, 16
NCORES = 8
LCH = L // NCORES              # l-chunk per core (256)
T = B * LCH                    # tokens per core (1024), b-major
QCH = 512                      # token-chunk (PSUM bank width in fp32)
NQ = T // QCH                  # 2
NJ = HD // 128                 # 32 h-tiles
NK = D // 128                  # 4 d-tiles
CW = 4                         # w_in/w_out column chunks (1024 cols each)
JPC = NJ // CW                 # 8 j-tiles per chunk

_cache = {}


def _build():
    from concourse import bacc, bass, mybir, tile

    F32 = mybir.dt.float32
    BF16 = mybir.dt.bfloat16
    PSUM = bass.MemorySpace.PSUM

    nc = bacc.Bacc("TRN2", target_bir_lowering=False, debug=False)

    xT_d = nc.dram_tensor("xT", [D, T], BF16, kind="ExternalInput")
    w_in_d = nc.dram_tensor("w_in", [D, HD], BF16, kind="ExternalInput")
    w_out_d = nc.dram_tensor("w_out", [HD, D], BF16, kind="ExternalInput")
    # host-computed mod table, laid out [128 h-part, j-tile, l]
    msb_d = nc.dram_tensor("msb", [128, NJ * LCH], BF16, kind="ExternalInput")
    yT_d = nc.dram_tensor("yT", [D, T], BF16, kind="ExternalOutput")

    with tile.TileContext(nc) as tc:
        with (
            tc.tile_pool(name="win", bufs=1) as winp,
            tc.tile_pool(name="wout", bufs=1) as woutp,
            tc.tile_pool(name="xts", bufs=1) as xtp,
            tc.tile_pool(name="ms", bufs=1) as msp,
            tc.tile_pool(name="hm", bufs=4) as hmp,
            tc.tile_pool(name="yo", bufs=2) as yop,
            tc.tile_pool(name="pa", bufs=4, space=PSUM) as pap,
            tc.tile_pool(name="py", bufs=4, space=PSUM) as pyp,
        ):
            w_in_r = w_in_d.ap().rearrange("(k p) (c h) -> c p k h", p=128, c=CW)
            xT_r = xT_d.ap().rearrange("(k p) (q t) -> q p k t", p=128, q=NQ)
            w_out_r = w_out_d.ap().rearrange(
                "(g jj p) i -> g p jj i", p=128, jj=JPC
            )
            yT_r = yT_d.ap().rearrange("(j2 p) (q t) -> q p j2 t", p=128, q=NQ)
            msb_r = msb_d.ap()

            win_c = [
                winp.tile([128, NK, 1024], BF16, name=f"win{c}", tag=f"win{c}")
                for c in range(CW)
            ]
            wout_g = [
                woutp.tile([128, JPC, D], BF16, name=f"wout{g}", tag=f"wout{g}")
                for g in range(CW)
            ]
            xts_q = [
                xtp.tile([128, NK, QCH], BF16, name=f"xts{q}", tag=f"xts{q}")
                for q in range(NQ)
            ]
            msb = msp.tile([128, NJ * LCH], BF16, tag="msb")

            # ---- DMA issue order: two HWDGE rings, just-in-time.
            # ring A (sync) and ring B (scalar) both start with the tiny
            # pieces the first pa/py groups need, then stream the rest.
            rA, rB = nc.sync, nc.scalar
            rA.dma_start(xts_q[0][:, 0:1, :], xT_r[0][:, 0:1, :])
            rB.dma_start(win_c[0][:, :, 0:128], w_in_r[0][:, :, 0:128])
            rA.dma_start(xts_q[0][:, 1:2, :], xT_r[0][:, 1:2, :])
            rB.dma_start(win_c[0][:, :, 128:512], w_in_r[0][:, :, 128:512])
            rA.dma_start(xts_q[0][:, 2:4, :], xT_r[0][:, 2:4, :])
            rA.dma_start(msb[:, 0 : 4 * LCH], msb_r[:, 0 : 4 * LCH])
            rB.dma_start(win_c[0][:, :, 512:1024], w_in_r[0][:, :, 512:1024])
            rA.dma_start(wout_g[0][:, 0:2, :], w_out_r[0][:, 0:2, :])
            rA.dma_start(wout_g[0][:, 2:JPC, :], w_out_r[0][:, 2:JPC, :])
            rB.dma_start(msb[:, 4 * LCH : 12 * LCH], msb_r[:, 4 * LCH : 12 * LCH])
            rB.dma_start(win_c[1][:], w_in_r[1])
            rA.dma_start(wout_g[1][:], w_out_r[1])
            rB.dma_start(win_c[2][:], w_in_r[2])
            rA.dma_start(msb[:, 12 * LCH : 20 * LCH], msb_r[:, 12 * LCH : 20 * LCH])
            rA.dma_start(wout_g[2][:], w_out_r[2])
            rB.dma_start(win_c[3][:], w_in_r[3])
            rA.dma_start(xts_q[1][:], xT_r[1])
            rB.dma_start(msb[:, 20 * LCH : 32 * LCH], msb_r[:, 20 * LCH : 32 * LCH])
            rB.dma_start(wout_g[3][:], w_out_r[3])

            # ---- fused main loop (py stage software-pipelined by two j,
            # so PE never waits on the vector-engine modulate) ----
            for q in range(NQ):
                pys = [pyp.tile([128, QCH], F32, name=f"py{q}_{j2}", tag="py")
                       for j2 in range(NK)]

                def emit_py(phm, pj):
                    for j2 in range(NK):
                        nc.tensor.matmul(
                            pys[j2][:],
                            wout_g[pj // JPC][:, pj % JPC,
                                              128 * j2 : 128 * (j2 + 1)],
                            phm[:],
                            start=(pj == 0),
                            stop=(pj == NJ - 1),
                        )

                pend = []
                for j in range(NJ):
                    c, jc = j // JPC, j % JPC
                    pa = pap.tile([128, QCH], F32, tag="pa")
                    for k in range(NK):
                        nc.tensor.matmul(
                            pa[:],
                            win_c[c][:, k, 128 * jc : 128 * (jc + 1)],
                            xts_q[q][:, k, :],
                            start=(k == 0),
                            stop=(k == NK - 1),
                        )
                    ms = msb[:, LCH * j : LCH * (j + 1)]
                    hm = hmp.tile([128, QCH], BF16, tag="hm")
                    nc.vector.tensor_mul(hm[:, 0:LCH], pa[:, 0:LCH], ms)
                    nc.vector.tensor_mul(hm[:, LCH:QCH], pa[:, LCH:QCH], ms)
                    pend.append((hm, j))
                    if len(pend) > 2:
                        emit_py(*pend.pop(0))
                for phm, pj in pend:
                    emit_py(phm, pj)
                # evictions alternate scalar/vector; each store leaves on
                # both HWDGE rings (partition halves) to halve the
                # descriptor storm at the tail
                yo = yop.tile([128, NK, QCH], BF16, tag="yo")
                for j2 in range(NK):
                    if j2 % 2 == 0:
                        nc.scalar.copy(yo[:, j2, :], pys[j2][:])
                    else:
                        nc.vector.tensor_copy(yo[:, j2, :], pys[j2][:])
                    rA.dma_start(yT_r[q][0:64, j2 : j2 + 1, :],
                                 yo[0:64, j2 : j2 + 1, :])
                    rB.dma_start(yT_r[q][64:128, j2 : j2 + 1, :],
                                 yo[64:128, j2 : j2 + 1, :])

    nc.finalize()
    return nc


def _get_nc():
    if "nc" not in _cache:
        _cache["nc"] = _build()
    return _cache["nc"]


def _bf(a):
    import ml_dtypes
    return np.ascontiguousarray(np.asarray(a, dtype=np.float32).astype(ml_dtypes.bfloat16))


def _in_maps(x, input_proj, output_proj, floquet_energies, drive_weights,
             coupling_matrix):
    coupled = coupling_matrix.astype(np.float64) @ drive_weights.astype(np.float64)
    E = floquet_energies.astype(np.float64)
    a_coef = coupled[:, None] * np.cos(E)          # [M, HD]
    b_coef = -coupled[:, None] * np.sin(E)         # [M, HD]
    t = np.arange(L, dtype=np.float64) / L
    harm = np.arange(1, M + 1, dtype=np.float64)
    ang = 2.0 * np.pi * harm[None, :] * t[:, None]  # [L, M]
    mod = np.cos(ang) @ a_coef + np.sin(ang) @ b_coef  # [L, HD]

    w_in = _bf(input_proj)
    w_out = _bf(output_proj)

    maps = []
    for c in range(NCORES):
        mc = mod[c * LCH : (c + 1) * LCH, :]        # [LCH, HD]
        # msb[p, LCH*j + l] = mod[l, 128*j + p]
        msb = np.ascontiguousarray(
            mc.T.reshape(NJ, 128, LCH).transpose(1, 0, 2).reshape(128, NJ * LCH)
        )
        # xT[d, b*LCH + l] = x[b, c*LCH + l, d]
        xc = x[:, c * LCH : (c + 1) * LCH, :]       # [B, LCH, D]
        xT = _bf(xc.transpose(2, 0, 1).reshape(D, T))
        maps.append({
            "xT": xT,
            "w_in": w_in,
            "w_out": w_out,
            "msb": _bf(msb),
        })
    return maps


def kernel(x, input_proj, output_proj, floquet_energies, drive_weights,
           coupling_matrix, _trace=False, _trace_kwargs=None):
    from concourse.bass_utils import run_bass_kernel_spmd

    nc = _get_nc()
    maps = _in_maps(x, input_proj, output_proj, floquet_energies,
                    drive_weights, coupling_matrix)
    kw = dict(_trace_kwargs or {})
    res = run_bass_kernel_spmd(nc, maps, list(range(NCORES)), trace=_trace, **kw)
    out = np.empty((B, L, D), dtype=np.float32)
    for c in range(NCORES):
        yT = np.asarray(res.results[c]["yT"], dtype=np.float32)  # [D, T]
        out[:, c * LCH : (c + 1) * LCH, :] = yT.reshape(D, B, LCH).transpose(1, 2, 0)
    if _trace:
        return out, res
    return out


# revision 5
# speedup vs baseline: 1.0079x; 1.0079x over previous
"""HDTimeCrystalBlock kernel for 8 Trainium2 NeuronCores.

Math: out = ((x @ W_in) * mod[None]) @ W_out, where
  mod[l,h] = sum_m coupled[m] * cos(omega*(m+1)*t[l] + E[m,h])

mod depends only on (l,h) -- a [L,HD] table -- and costs ~0.5 GFLOP to
build, so it is computed on the HOST in fp64 (cos/sin tables + two tiny
[L,32]@[32,HD] gemms) and shipped to the device as a 2 MiB bf16 input.
The device is then a pure matmul pipeline: pa = W_in^T-tile @ x-tile,
hm = pa * mod-tile (DVE), y += W_out-tile^T @ hm -- no mod matmuls, no
PSUM->SBUF mod copies, no warm-up filler. The PE starts as soon as the
first x / W_in strips land (~7us) and streams 512 back-to-back
[128x128]@[128x512] bf16 matmuls (~110us, the PE roofline).

Sharding: split L=2048 into 8 chunks of 256; each core handles its
l-chunk for ALL 4 batches (1024 tokens, b-major). mod depends only on
l, so each core loads only its own [LCH,HD] mod slice. Activations stay
transposed ([feature, token]); weights are the stationary operand.

DMA: two HWDGE rings (sync + scalar). Early transfers are small and
ordered just-in-time (xts0 per-k, w_in j0 strip first) so the first pa
group issues ~1.3us after engine start; later transfers are big. Output
stores are split across both rings by partition halves to halve the
descriptor-storm tail.
"""
import math

import numpy as np

B, L, D, HD, M = 4, 2048, 512, 4096, 16
NCORES = 8
LCH = L // NCORES              # l-chunk per core (256)
T = B * LCH                    # tokens per core (1024), b-major
QCH = 512                      # token-chunk (PSUM bank width in fp32)
NQ = T // QCH                  # 2
NJ = HD // 128                 # 32 h-tiles
NK = D // 128                  # 4 d-tiles
CW = 4                         # w_in/w_out column chunks (1024 cols each)
JPC = NJ // CW                 # 8 j-tiles per chunk

_cache = {}


def _build():
    from concourse import bacc, bass, mybir, tile

    F32 = mybir.dt.float32
    BF16 = mybir.dt.bfloat16
    PSUM = bass.MemorySpace.PSUM

    nc = bacc.Bacc("TRN2", target_bir_lowering=False, debug=False)

    xT_d = nc.dram_tensor("xT", [D, T], BF16, kind="ExternalInput")
    w_in_d = nc.dram_tensor("w_in", [D, HD], BF16, kind="ExternalInput")
    w_out_d = nc.dram_tensor("w_out", [HD, D], BF16, kind="ExternalInput")
    # host-computed mod table, laid out [128 h-part, j-tile, l]
    msb_d = nc.dram_tensor("msb", [128, NJ * LCH], BF16, kind="ExternalInput")
    yT_d = nc.dram_tensor("yT", [D, T], BF16, kind="ExternalOutput")

    with tile.TileContext(nc) as tc:
        with (
            tc.tile_pool(name="win", bufs=1) as winp,
            tc.tile_pool(name="wout", bufs=1) as woutp,
            tc.tile_pool(name="xts", bufs=1) as xtp,
            tc.tile_pool(name="ms", bufs=1) as msp,
            tc.tile_pool(name="hm", bufs=4) as hmp,
            tc.tile_pool(name="yo", bufs=2) as yop,
            tc.tile_pool(name="pa", bufs=4, space=PSUM) as pap,
            tc.tile_pool(name="py", bufs=4, space=PSUM) as pyp,
        ):
            w_in_r = w_in_d.ap().rearrange("(k p) (c h) -> c p k h", p=128, c=CW)
            xT_r = xT_d.ap().rearrange("(k p) (q t) -> q p k t", p=128, q=NQ)
            w_out_r = w_out_d.ap().rearrange(
                "(g jj p) i -> g p jj i", p=128, jj=JPC
            )
            yT_r = yT_d.ap().rearrange("(j2 p) (q t) -> q p j2 t", p=128, q=NQ)
            msb_r = msb_d.ap()

            win_c = [
                winp.tile([128, NK, 1024], BF16, name=f"win{c}", tag=f"win{c}")
                for c in range(CW)
            ]
            wout_g = [
                woutp.tile([128, JPC, D], BF16, name=f"wout{g}", tag=f"wout{g}")
                for g in range(CW)
            ]
            xts_q = [
                xtp.tile([128, NK, QCH], BF16, name=f"xts{q}", tag=f"xts{q}")
                for q in range(NQ)
            ]
            msb = msp.tile([128, NJ * LCH], BF16, tag="msb")

            # ---- DMA issue order: two HWDGE rings, just-in-time.
            # Early transfers are k-slices ([128, 1, ...]: 1-2 KB per
            # partition line -> big descriptors, fast landing) so the
            # first pa matmuls can start k-by-k as data arrives.
            rA, rB = nc.sync, nc.scalar
            rA.dma_start(xts_q[0][:, 0:1, :], xT_r[0][:, 0:1, :])
            rB.dma_start(win_c[0][:, 0:1, :], w_in_r[0][:, 0:1, :])
            rA.dma_start(xts_q[0][:, 1:2, :], xT_r[0][:, 1:2, :])
            rB.dma_start(win_c[0][:, 1:2, :], w_in_r[0][:, 1:2, :])
            rA.dma_start(msb[:, 0 : 4 * LCH], msb_r[:, 0 : 4 * LCH])
            rB.dma_start(win_c[0][:, 2:4, :], w_in_r[0][:, 2:4, :])
            rA.dma_start(wout_g[0][:, 0:2, :], w_out_r[0][:, 0:2, :])
            rA.dma_start(xts_q[0][:, 2:4, :], xT_r[0][:, 2:4, :])
            rB.dma_start(msb[:, 4 * LCH : 12 * LCH], msb_r[:, 4 * LCH : 12 * LCH])
            rA.dma_start(wout_g[0][:, 2:JPC, :], w_out_r[0][:, 2:JPC, :])
            rB.dma_start(win_c[1][:], w_in_r[1])
            rA.dma_start(wout_g[1][:], w_out_r[1])
            rB.dma_start(win_c[2][:], w_in_r[2])
            rA.dma_start(xts_q[1][:], xT_r[1])
            rB.dma_start(msb[:, 12 * LCH : 20 * LCH], msb_r[:, 12 * LCH : 20 * LCH])
            rA.dma_start(wout_g[2][:], w_out_r[2])
            rB.dma_start(win_c[3][:], w_in_r[3])
            rA.dma_start(wout_g[3][:], w_out_r[3])
            rB.dma_start(msb[:, 20 * LCH : 32 * LCH], msb_r[:, 20 * LCH : 32 * LCH])

            # ---- fused main loop (py stage software-pipelined by two j,
            # so PE never waits on the vector-engine modulate) ----
            for q in range(NQ):
                pys = [pyp.tile([128, QCH], F32, name=f"py{q}_{j2}", tag="py")
                       for j2 in range(NK)]

                def emit_py(phm, pj):
                    for j2 in range(NK):
                        nc.tensor.matmul(
                            pys[j2][:],
                            wout_g[pj // JPC][:, pj % JPC,
                                              128 * j2 : 128 * (j2 + 1)],
                            phm[:],
                            start=(pj == 0),
                            stop=(pj == NJ - 1),
                        )

                pend = []
                jstart = 0
                if q == 0:
                    # k-major warm-up block: j0..j3 accumulate k-by-k so
                    # the PE starts as soon as the first (k-slice of x,
                    # k-slice of w_in) pair lands, instead of waiting for
                    # all four k-slices
                    jstart = 4
                    pas = [pap.tile([128, QCH], F32, name=f"pas{jj}",
                                    tag="pa")
                           for jj in range(4)]
                    for k in range(NK):
                        for j in range(4):
                            nc.tensor.matmul(
                                pas[j][:],
                                win_c[0][:, k, 128 * j : 128 * (j + 1)],
                                xts_q[0][:, k, :],
                                start=(k == 0),
                                stop=(k == NK - 1),
                            )
                    for j in range(4):
                        ms = msb[:, LCH * j : LCH * (j + 1)]
                        hm = hmp.tile([128, QCH], BF16, tag="hm")
                        nc.vector.tensor_mul(hm[:, 0:LCH], pas[j][:, 0:LCH], ms)
                        nc.vector.tensor_mul(hm[:, LCH:QCH], pas[j][:, LCH:QCH], ms)
                        pend.append((hm, j))
                        if len(pend) > 2:
                            emit_py(*pend.pop(0))
                for j in range(jstart, NJ):
                    c, jc = j // JPC, j % JPC
                    pa = pap.tile([128, QCH], F32, tag="pa")
                    for k in range(NK):
                        nc.tensor.matmul(
                            pa[:],
                            win_c[c][:, k, 128 * jc : 128 * (jc + 1)],
                            xts_q[q][:, k, :],
                            start=(k == 0),
                            stop=(k == NK - 1),
                        )
                    ms = msb[:, LCH * j : LCH * (j + 1)]
                    hm = hmp.tile([128, QCH], BF16, tag="hm")
                    nc.vector.tensor_mul(hm[:, 0:LCH], pa[:, 0:LCH], ms)
                    nc.vector.tensor_mul(hm[:, LCH:QCH], pa[:, LCH:QCH], ms)
                    pend.append((hm, j))
                    if len(pend) > 2:
                        emit_py(*pend.pop(0))
                for phm, pj in pend:
                    emit_py(phm, pj)
                # evictions alternate scalar/vector; each store leaves on
                # both HWDGE rings (partition halves) to halve the
                # descriptor storm at the tail
                yo = yop.tile([128, NK, QCH], BF16, tag="yo")
                for j2 in range(NK):
                    if j2 % 2 == 0:
                        nc.scalar.copy(yo[:, j2, :], pys[j2][:])
                    else:
                        nc.vector.tensor_copy(yo[:, j2, :], pys[j2][:])
                    rA.dma_start(yT_r[q][0:64, j2 : j2 + 1, :],
                                 yo[0:64, j2 : j2 + 1, :])
                    rB.dma_start(yT_r[q][64:128, j2 : j2 + 1, :],
                                 yo[64:128, j2 : j2 + 1, :])

    nc.finalize()
    return nc


def _get_nc():
    if "nc" not in _cache:
        _cache["nc"] = _build()
    return _cache["nc"]


def _bf(a):
    import ml_dtypes
    return np.ascontiguousarray(np.asarray(a, dtype=np.float32).astype(ml_dtypes.bfloat16))


def _in_maps(x, input_proj, output_proj, floquet_energies, drive_weights,
             coupling_matrix):
    coupled = coupling_matrix.astype(np.float64) @ drive_weights.astype(np.float64)
    E = floquet_energies.astype(np.float64)
    a_coef = coupled[:, None] * np.cos(E)          # [M, HD]
    b_coef = -coupled[:, None] * np.sin(E)         # [M, HD]
    t = np.arange(L, dtype=np.float64) / L
    harm = np.arange(1, M + 1, dtype=np.float64)
    ang = 2.0 * np.pi * harm[None, :] * t[:, None]  # [L, M]
    mod = np.cos(ang) @ a_coef + np.sin(ang) @ b_coef  # [L, HD]

    w_in = _bf(input_proj)
    w_out = _bf(output_proj)

    maps = []
    for c in range(NCORES):
        mc = mod[c * LCH : (c + 1) * LCH, :]        # [LCH, HD]
        # msb[p, LCH*j + l] = mod[l, 128*j + p]
        msb = np.ascontiguousarray(
            mc.T.reshape(NJ, 128, LCH).transpose(1, 0, 2).reshape(128, NJ * LCH)
        )
        # xT[d, b*LCH + l] = x[b, c*LCH + l, d]
        xc = x[:, c * LCH : (c + 1) * LCH, :]       # [B, LCH, D]
        xT = _bf(xc.transpose(2, 0, 1).reshape(D, T))
        maps.append({
            "xT": xT,
            "w_in": w_in,
            "w_out": w_out,
            "msb": _bf(msb),
        })
    return maps


def kernel(x, input_proj, output_proj, floquet_energies, drive_weights,
           coupling_matrix, _trace=False, _trace_kwargs=None):
    from concourse.bass_utils import run_bass_kernel_spmd

    nc = _get_nc()
    maps = _in_maps(x, input_proj, output_proj, floquet_energies,
                    drive_weights, coupling_matrix)
    kw = dict(_trace_kwargs or {})
    res = run_bass_kernel_spmd(nc, maps, list(range(NCORES)), trace=_trace, **kw)
    out = np.empty((B, L, D), dtype=np.float32)
    for c in range(NCORES):
        yT = np.asarray(res.results[c]["yT"], dtype=np.float32)  # [D, T]
        out[:, c * LCH : (c + 1) * LCH, :] = yT.reshape(D, B, LCH).transpose(1, 2, 0)
    if _trace:
        return out, res
    return out
